# revision 29
# baseline (speedup 1.0000x reference)
"""Trainium2 Bass kernel for per-sample channel attention.

Computation (per batch sample):
    x: (C=512, N=4096) fp32
    energy = x @ x.T                       (C, C), symmetric
    m_j = min_i energy[i, j]               (column min == row min by symmetry)
    A[i, j] = exp(m_j - energy[i, j]) / sum_i exp(m_j - energy[i, j])
    out = gamma * (A @ x) + x

Sharding: data-parallel over the batch axis, 2 samples per NeuronCore on 8
cores.  Each core runs an identical program on its own slice.

Layout tricks:
  * energy is symmetric, so an energy tile held as [rows j (partitions),
    cols i (free)] is simultaneously A^T's pre-softmax input with the
    softmax reduction running along the *free* axis; the normalized tile is
    directly the lhsT (= A^T) of the second matmul.  Only the upper block
    triangle of energy is computed; missing blocks are PE-transposes of the
    mirrored blocks.
  * both matmuls run in bf16 (full PE rate; fp32 would be 4x slower).  x is
    cast to bf16 once — the cast feeds the PE transposes (xT for the Gram
    matmul) and is reused as the rhs of the second matmul.  The softmax
    itself runs in fp32 (energy accumulates in fp32 PSUM).
  * gamma and the softmax 1/sum are folded into A^T, so the epilogue is a
    single vector add of the exact fp32 residual x.
  * with gamma == 0 the attention branch contributes exactly 0 and the
    output equals the input bit-for-bit.
"""

import numpy as np

import concourse.bass as bass
import concourse.mybir as mybir
import concourse.tile as tile
from concourse import bass_utils
from concourse.bass import ds, ts
from concourse.masks import make_identity

B, C, HH, WW = 16, 512, 64, 64
N = HH * WW            # 4096
NCORES = 8
B_LOC = B // NCORES    # 2 samples per core
P = 128
CT = C // P            # 4 channel tiles
KT = N // P            # 32 contraction tiles for the Gram matmul
NCH = N // 512         # 8 output chunks of 512 along N


def _split_multi_waits(nc: bass.Bass) -> bass.Bass:
    """The walrus build in this container rejects more than one semaphore
    wait command per instruction.  Tile's scheduler freely attaches several
    waits to one instruction (and its kernel-tail drain aggregates waits for
    every outstanding semaphore).  Move the extra waits onto preceding NoOps
    on the same engine — semantically identical, since all waits complete
    before the instruction issues either way."""
    for f in nc.m.functions:
        for blk in f.blocks:
            out = []
            changed = False
            for inst in blk.instructions:
                si = inst.sync_info
                if si is not None and len(si.on_wait) > 1:
                    changed = True
                    waits = list(si.on_wait)
                    for i, wt in enumerate(waits[:-1]):
                        out.append(
                            mybir.InstNoOp(
                                name=f"{inst.name}-w{i}",
                                engine=inst.engine,
                                sync_info=mybir.SyncInfo(on_wait=[wt], on_update=[]),
                                bass_nofuse=True,
                            )
                        )
                    inst.sync_info = mybir.SyncInfo(
                        on_wait=[waits[-1]], on_update=list(si.on_update)
                    )
                out.append(inst)
            if changed:
                blk.instructions = out
    return nc


def build_bass(rep: int = 1) -> bass.Bass:
    f32 = mybir.dt.float32
    bf16 = mybir.dt.bfloat16

    nc = bass.Bass(
        target_bir_lowering=False,
        trn_type="TRN2",
        debug=False,
        dynamic_dma_scratch_size=1024,
    )
    x_dram = nc.dram_tensor("inputs", [B_LOC, C, N], f32, kind="ExternalInput")
    g_dram = nc.dram_tensor("gamma", [1], f32, kind="ExternalInput")
    y_dram = nc.dram_tensor("out", [B_LOC, C, N], f32, kind="ExternalOutput")
    xap = x_dram.ap()
    yap = y_dram.ap()

    with tile.TileContext(nc) as tc:
        with (
            tc.tile_pool(name="xp", bufs=2) as xp,
            tc.tile_pool(name="xbfp", bufs=1) as xbfp,
            tc.tile_pool(name="xtp", bufs=1) as xtp,
            tc.tile_pool(name="wp", bufs=2) as wp,
            tc.tile_pool(name="xjp", bufs=8) as xjp,
            tc.tile_pool(name="obp", bufs=3) as obp,
            tc.tile_pool(name="esbp", bufs=1) as esbp,
            tc.tile_pool(name="consts", bufs=1) as consts,
            tc.tile_pool(name="small", bufs=4) as small,
            tc.tile_pool(name="tps", bufs=2, space="PSUM") as tps,
            tc.tile_pool(name="eps", bufs=2, space="PSUM") as eps,
            tc.tile_pool(name="ops", bufs=3, space="PSUM") as ops,
            tc.tile_pool(name="wps", bufs=1, space="PSUM") as wps,
        ):
            identb = consts.tile([P, P], bf16, tag="identb")
            make_identity(nc, identb)
            ident = consts.tile([P, P], f32, tag="ident")
            make_identity(nc, ident)
            gbc = consts.tile([P, 1], f32, tag="gbc")
            nc.sync.dma_start(out=gbc, in_=g_dram.ap().to_broadcast((P, 1)))

            for _ in range(rep):
                for b in range(B_LOC):
                    # ---- load x (natural [c, n] fp32) + bf16 working copy ----
                    # n-chunk-major order so the first transposes (which need
                    # the low-n slices of all four channel tiles) can start
                    # as early as possible
                    x = xp.tile([P, CT, N], f32, tag="x")
                    xb = xbfp.tile([P, CT, N], bf16, tag="xb")
                    for q in range(8):
                        for ct in range(CT):
                            nc.sync.dma_start(
                                out=x[:, ct, ds(q * 512, 512)],
                                in_=xap[b, ts(ct, P), ds(q * 512, 512)],
                            )
                            nc.vector.tensor_copy(
                                xb[:, ct, ds(q * 512, 512)],
                                x[:, ct, ds(q * 512, 512)],
                            )

                    # ---- transpose xb -> xT [n, c] bf16 (PE + ACT evac) ----
                    xT = xtp.tile([P, KT, C], bf16, tag="xT")
                    for k in range(KT):
                        tp = tps.tile([P, C], bf16, tag="tp")
                        for ct in range(CT):
                            nc.tensor.transpose(
                                tp[:, ts(ct, P)], xb[:, ct, ts(k, P)], identb
                            )
                        nc.scalar.copy(out=xT[:, k, :], in_=tp)

                    # ---- energy (upper block triangle) + softmax ----
                    w = wp.tile([P, CT, C], bf16, tag="w")
                    esb = {}
                    for mt in range(CT):
                        width = C - 128 * mt
                        ep = eps.tile([P, C], f32, tag="ep")
                        for k in range(KT):
                            nc.tensor.matmul(
                                ep[:, ds(128 * mt, width)],
                                xT[:, k, ts(mt, P)],
                                xT[:, k, ds(128 * mt, width)],
                                start=(k == 0),
                                stop=(k == KT - 1),
                            )
                        # fill the missing lower blocks: block (mt, bt<mt) is
                        # the transpose of block (bt, mt), evacuated to SBUF
                        # when row tile bt was processed
                        for bt in range(mt):
                            nc.tensor.transpose(
                                ep[:, ts(bt, P)],
                                esb[bt][:, ds((mt - bt - 1) * 128, 128)],
                                ident,
                            )
                        if mt < CT - 1:
                            esb[mt] = esbp.tile(
                                [P, C - 128 * (mt + 1)],
                                f32,
                                name=f"esb{mt}",
                                tag=f"esb{mt}",
                            )
                            nc.scalar.copy(
                                out=esb[mt],
                                in_=ep[:, ds(128 * (mt + 1), C - 128 * (mt + 1))],
                            )
                        mrow = small.tile([P, 1], f32, tag="mrow")
                        nc.vector.tensor_reduce(
                            mrow, ep, axis=mybir.AxisListType.X, op=mybir.AluOpType.min
                        )
                        ssum = small.tile([P, 1], f32, tag="ssum")
                        wtmp = wps.tile([P, C], f32, tag="wtmp")
                        nc.scalar.activation(
                            wtmp,
                            ep,
                            mybir.ActivationFunctionType.Exp,
                            bias=mrow,
                            scale=-1.0,
                            accum_out=ssum,
                        )
                        rg = small.tile([P, 1], f32, tag="rg")
                        nc.vector.reciprocal(rg, ssum)
                        rg2 = small.tile([P, 1], f32, tag="rg2")
                        nc.vector.tensor_mul(rg2, rg, gbc)
                        nc.vector.tensor_scalar_mul(w[:, mt, :], wtmp, rg2)

                    # ---- out = w^T-matmul + exact fp32 residual ----
                    # The rhs chunks are just-in-time bf16 casts from the
                    # exact x (alternating DVE/ACT), so xb's last reader is
                    # the transpose phase and the next batch's cast can
                    # overlap this batch's matmuls.  The residual add runs
                    # in-place in PSUM and the store DMA reads PSUM directly.
                    for chk in range(NCH):
                        xj = []
                        for jt in range(CT):
                            xc = xjp.tile([P, 512], bf16, name=f"xj{jt}", tag="xj")
                            nc.scalar.copy(out=xc, in_=x[:, jt, ds(chk * 512, 512)])
                            xj.append(xc)
                        for it in range(CT):
                            o = ops.tile([P, 512], f32, tag="o")
                            for jt in range(CT):
                                nc.tensor.matmul(
                                    o,
                                    w[:, jt, ts(it, P)],
                                    xj[jt],
                                    start=(jt == 0),
                                    stop=(jt == CT - 1),
                                )
                            ob = obp.tile([P, 512], f32, tag="ob")
                            nc.vector.tensor_add(ob, o, x[:, it, ds(chk * 512, 512)])
                            nc.sync.dma_start(
                                out=yap[b, ts(it, P), ds(chk * 512, 512)], in_=ob
                            )
    _split_multi_waits(nc)
    return nc


_NC_CACHE: dict = {}


def get_nc(rep: int = 1) -> bass.Bass:
    if rep not in _NC_CACHE:
        _NC_CACHE[rep] = build_bass(rep)
    return _NC_CACHE[rep]


def make_in_maps(inputs: np.ndarray, gamma: np.ndarray):
    x = np.ascontiguousarray(inputs, dtype=np.float32).reshape(NCORES, B_LOC, C, N)
    g = np.ascontiguousarray(gamma, dtype=np.float32).reshape(1)
    return [{"inputs": x[k], "gamma": g} for k in range(NCORES)]


def kernel(inputs: np.ndarray, gamma: np.ndarray) -> np.ndarray:
    assert inputs.shape == (B, C, HH, WW), inputs.shape
    in_maps = make_in_maps(inputs, gamma)
    res = bass_utils.run_bass_kernel_spmd(
        get_nc(), in_maps, core_ids=list(range(NCORES))
    )
    out = np.stack([r["out"] for r in res.results], axis=0)
    return out.reshape(B, C, HH, WW).astype(np.float32)
